# revision 1
# baseline (speedup 1.0000x reference)
"""Dilated sliding-window attention (WIN=5, DIL=2) Trainium2 Bass kernel.

Math: the reference scatters banded scores c_w[i] = Q_i . K_{i+off_w}
(off in {-4,-2,0,2,4}) into a zero S x S matrix and softmaxes the FULL
row, so off-band entries contribute exp(0)=1 each.  Closed form:

  out_i = (sumV + sum_w (e_wi - 1) V_{i+off_w}) / (S + sum_w (e_wi - 1))
  e_wi  = exp(c_wi) for in-range offsets, 1 otherwise (so e-1 drops out)

Sharding: 8 cores = 2 batches x 4 sequence shards of 1024 rows, each with
a 4-row halo on both sides (zero-padded at batch edges).  x is shipped
transposed ([E, rows]) and cast to bf16 on the host; all heavy matmuls run
in bf16 with fp32 PSUM accumulation.  Each core returns num (64,1024),
z (1,1024) and its partial V-sum; the host applies the tiny closed-form
epilogue (one fused multiply-add per output element) and unshards.
"""

import numpy as np

B, S, E = 2, 4096, 1024
QD = 64
WIN, DIL = 5, 2
HALF = WIN // 2
OFFS = [DIL * (w - HALF) for w in range(WIN)]  # [-4,-2,0,2,4]
H = HALF * DIL          # 4 halo rows each side
NC_ = 8                 # cores
SH = 4                  # seq shards per batch
R = S // SH             # 1024 own rows per core
RH = R + 2 * H          # 1032 rows incl. halo
RP = 1040               # padded row count (DMA-friendly)
NCHUNK = E // 128       # 8 contraction chunks
CT = (512, 512, 8)      # projection col-tiles covering [0, 1032)
NBT = R // 512          # 2 band col-tiles over own rows

_prog = None


def _build_program():
    """Build + compile the SPMD Bass program once."""
    from contextlib import ExitStack
    import concourse.bass as bass
    import concourse.tile as tile
    from concourse import bacc, mybir

    F32 = mybir.dt.float32
    BF16 = mybir.dt.bfloat16
    AF = mybir.ActivationFunctionType
    OP = mybir.AluOpType

    nc = bacc.Bacc("TRN2", target_bir_lowering=False, debug=False,
                   enable_asserts=False)

    xt = nc.dram_tensor("xt", [E, RP], BF16, kind="ExternalInput").ap()
    wq = nc.dram_tensor("wq", [128, NCHUNK * QD], BF16, kind="ExternalInput").ap()
    wk = nc.dram_tensor("wk", [128, NCHUNK * QD], BF16, kind="ExternalInput").ap()
    wv = nc.dram_tensor("wv", [128, NCHUNK * QD], BF16, kind="ExternalInput").ap()
    bias3 = nc.dram_tensor("bias3", [QD, 3], F32, kind="ExternalInput").ap()
    num_d = nc.dram_tensor("num", [QD, R], F32, kind="ExternalOutput").ap()
    z_d = nc.dram_tensor("z", [1, R], F32, kind="ExternalOutput").ap()
    psumv_d = nc.dram_tensor("psumv", [QD, 1], F32, kind="ExternalOutput").ap()

    with tile.TileContext(nc) as tc, ExitStack() as ctx:
        const = ctx.enter_context(tc.tile_pool(name="const", bufs=1))
        xpool = ctx.enter_context(tc.tile_pool(name="x", bufs=NCHUNK))
        qkv = ctx.enter_context(tc.tile_pool(name="qkv", bufs=1))
        bpool = ctx.enter_context(tc.tile_pool(name="band", bufs=2))
        epool = ctx.enter_context(tc.tile_pool(name="e", bufs=2))
        opool = ctx.enter_context(tc.tile_pool(name="out", bufs=2))
        pp = ctx.enter_context(tc.tile_pool(name="pp", bufs=3, space="PSUM"))
        pc = ctx.enter_context(tc.tile_pool(name="pc", bufs=2, space="PSUM"))
        pb = ctx.enter_context(tc.tile_pool(name="pb", bufs=2, space="PSUM"))

        # ---- constant / weight loads ----
        w_sb = {}
        for name, dram in (("q", wq), ("k", wk), ("v", wv)):
            t = const.tile([128, NCHUNK * QD], BF16, tag=f"w{name}")
            nc.sync.dma_start(t[:], dram[:])
            w_sb[name] = t
        bias_sb = const.tile([QD, 3], F32, tag="bias")
        nc.sync.dma_start(bias_sb[:], bias3[:])
        ones_col = const.tile([QD, 1], BF16, tag="onesc")
        nc.vector.memset(ones_col[:], 1.0)
        ones_row = const.tile([1, QD], BF16, tag="onesr")
        nc.vector.memset(ones_row[:], 1.0)

        # ---- x chunk loads ----
        xch = []
        for k in range(NCHUNK):
            t = xpool.tile([128, RP], BF16, tag="xch")
            nc.sync.dma_start(t[:], xt[k * 128:(k + 1) * 128, :])
            xch.append(t)

        # ---- stage A: projections qt/kt/vt = W_chunk^T @ xT_chunk ----
        qt = qkv.tile([QD, RH], BF16, tag="qt")
        kt = qkv.tile([QD, RH], BF16, tag="kt")
        vt = qkv.tile([QD, RH], BF16, tag="vt")
        dest = {"q": qt, "k": kt, "v": vt}
        col = 0
        for ct_n in CT:
            for pi, pname in enumerate(("q", "k", "v")):
                pt = pp.tile([QD, 512], F32, tag="pp")
                for k in range(NCHUNK):
                    nc.tensor.matmul(
                        pt[:, :ct_n],
                        lhsT=w_sb[pname][:, k * QD:(k + 1) * QD],
                        rhs=xch[k][:, col:col + ct_n],
                        start=(k == 0), stop=(k == NCHUNK - 1),
                    )
                # PSUM -> SBUF with bias add, cast to bf16
                nc.scalar.activation(dest[pname][:, col:col + ct_n],
                                     pt[:, :ct_n], AF.Identity,
                                     bias=bias_sb[:, pi:pi + 1], scale=1.0)
            col += ct_n

        # ---- psumv: per-core partial sum of V over own rows ----
        psumv_sb = opool.tile([QD, 1], F32, tag="psumv")
        nc.vector.tensor_reduce(psumv_sb[:], vt[:, H:H + R],
                                mybir.AxisListType.X, OP.add)
        nc.sync.dma_start(psumv_d[:], psumv_sb[:])

        # ---- stage B: band scores, exp, broadcast, V accumulation ----
        for bt in range(NBT):
            s0 = H + bt * 512          # own-col start in padded coords
            e_all = epool.tile([1, WIN * 512], BF16, tag="eall")
            for w, off in enumerate(OFFS):
                prod = bpool.tile([QD, 512], BF16, tag="prod")
                nc.vector.tensor_mul(prod[:], qt[:, s0:s0 + 512],
                                     kt[:, s0 + off:s0 + off + 512])
                cps = pc.tile([1, 512], F32, tag="cps")
                nc.tensor.matmul(cps[:], lhsT=ones_col[:], rhs=prod[:],
                                 start=True, stop=True)
                nc.scalar.activation(e_all[:, w * 512:(w + 1) * 512],
                                     cps[:], AF.Exp)

            # z = sum_w e_w  (pairwise bf16 adds, all-SBUF)
            za = bpool.tile([1, 2 * 512], BF16, tag="za")
            nc.vector.tensor_add(za[:, :512], e_all[:, 0:512],
                                 e_all[:, 512:1024])
            nc.vector.tensor_add(za[:, 512:1024], e_all[:, 1024:1536],
                                 e_all[:, 1536:2048])
            zb = bpool.tile([1, 512], BF16, tag="zb")
            nc.vector.tensor_add(zb[:], za[:, :512], za[:, 512:1024])
            z_sb = opool.tile([1, 512], F32, tag="zsb")
            nc.vector.tensor_add(z_sb[:], zb[:], e_all[:, 2048:2560])
            nc.sync.dma_start(z_d[:, bt * 512:(bt + 1) * 512], z_sb[:])

            # vsum5 = sum_w V_shift  (bf16 adds)
            va = bpool.tile([QD, 2 * 512], BF16, tag="va")
            nc.vector.tensor_add(va[:, :512], vt[:, s0 - 4:s0 + 508],
                                 vt[:, s0 - 2:s0 + 510])
            nc.vector.tensor_add(va[:, 512:1024], vt[:, s0:s0 + 512],
                                 vt[:, s0 + 2:s0 + 514])
            vb = bpool.tile([QD, 512], BF16, tag="vb")
            nc.vector.tensor_add(vb[:], va[:, :512], va[:, 512:1024])
            vsum5 = bpool.tile([QD, 512], BF16, tag="vsum5")
            nc.vector.tensor_add(vsum5[:], vb[:], vt[:, s0 + 4:s0 + 516])

            # num = sum_w e_w * V_shift - vsum5
            num_sb = opool.tile([QD, 512], F32, tag="numsb")
            tmps = []
            for w, off in enumerate(OFFS):
                ebc = pb.tile([QD, 512], F32, tag="ebc")
                nc.tensor.matmul(ebc[:], lhsT=ones_row[:],
                                 rhs=e_all[:, w * 512:(w + 1) * 512],
                                 start=True, stop=True)
                tmp = bpool.tile([QD, 512], BF16, tag=f"tmp{w % 2}")
                nc.vector.tensor_mul(tmp[:], ebc[:],
                                     vt[:, s0 + off:s0 + off + 512])
                tmps.append(tmp)
                if w == 1:
                    acc01 = bpool.tile([QD, 512], BF16, tag="acc01")
                    nc.vector.tensor_add(acc01[:], tmps[0][:], tmps[1][:])
                elif w == 3:
                    acc23 = bpool.tile([QD, 512], BF16, tag="acc23")
                    nc.vector.tensor_add(acc23[:], tmps[2][:], tmps[3][:])
            acc03 = bpool.tile([QD, 512], BF16, tag="acc03")
            nc.vector.tensor_add(acc03[:], acc01[:], acc23[:])
            acc04 = bpool.tile([QD, 512], BF16, tag="acc04")
            nc.vector.tensor_add(acc04[:], acc03[:], tmps[4][:])
            nc.vector.tensor_sub(num_sb[:], acc04[:], vsum5[:])
            nc.sync.dma_start(num_d[:, bt * 512:(bt + 1) * 512], num_sb[:])

    nc.compile()
    return nc


def _get_prog():
    global _prog
    if _prog is None:
        _prog = _build_program()
    return _prog


def _host_prep(x, Wq, bq, Wk, bk, Wv, bv):
    """Build the 8 per-core input maps."""
    import ml_dtypes
    bf16 = ml_dtypes.bfloat16

    def chunk_w(W):
        # [E, QD] -> [128, NCHUNK*QD] with chunk k at cols k*QD:(k+1)*QD
        return np.ascontiguousarray(
            W.reshape(NCHUNK, 128, QD).transpose(1, 0, 2).reshape(128, NCHUNK * QD)
        ).astype(bf16)

    wqc, wkc, wvc = chunk_w(Wq), chunk_w(Wk), chunk_w(Wv)
    bias3 = np.ascontiguousarray(
        np.stack([bq, bk, bv], axis=1).astype(np.float32))

    in_maps = []
    for c in range(NC_):
        b, sh = divmod(c, SH)
        r0 = sh * R
        lo, hi = r0 - H, r0 + R + H
        clo, chi = max(lo, 0), min(hi, S)
        pad = np.zeros((RP, E), np.float32)
        pad[clo - lo: clo - lo + (chi - clo), :] = x[b, clo:chi, :]
        xt = np.ascontiguousarray(pad.T).astype(bf16)
        in_maps.append({"xt": xt, "wq": wqc, "wk": wkc, "wv": wvc,
                        "bias3": bias3})
    return in_maps


def kernel(x, Wq, bq, Wk, bk, Wv, bv, _trace=False):
    from concourse import bass_utils

    x = np.asarray(x, np.float32)
    nc = _get_prog()
    in_maps = _host_prep(x, np.asarray(Wq), np.asarray(bq), np.asarray(Wk),
                         np.asarray(bk), np.asarray(Wv), np.asarray(bv))
    res = bass_utils.run_bass_kernel_spmd(
        nc, in_maps, core_ids=list(range(NC_)), trace=_trace)

    # host epilogue: out[i,:] = (num[:,i] + sumV_b) / (S - WIN + z[i])
    out = np.empty((B, S, QD), np.float32)
    sumv = np.zeros((B, QD), np.float64)
    for c in range(NC_):
        sumv[c // SH] += res.results[c]["psumv"][:, 0].astype(np.float64)
    for c in range(NC_):
        b, sh = divmod(c, SH)
        r = res.results[c]
        den = (S - WIN) + r["z"][0].astype(np.float64)  # S + sum(e-1)
        out[b, sh * R:(sh + 1) * R, :] = (
            (r["num"].T.astype(np.float64) + sumv[b][None, :]) / den[:, None]
        ).astype(np.float32)
    if _trace:
        kernel.last_exec_time_ns = res.exec_time_ns
        kernel.last_results = res
    return out
